# revision 1
# baseline (speedup 1.0000x reference)
"""Trainium2 Bass kernel for nn_BinaryPathEncoder.

Math: output row for position p is ones(256) pushed through a chain of
matrices P0/P1 chosen by the bits of p (LSB-first, topmost set bit dropped).
All distinct bit-paths form a complete binary tree with 2^17-1 nodes and
level k+1 of the tree is [P0 @ V_k, P1 @ V_k], so the whole tree costs
~17 GFLOP; each output row is then a gather from the tree table
(global row index = p-1).

Sharding: tree nodes (k>=3, m) are assigned to core m mod 8.  Children of
node (k, m) are (k+1, m) and (k+1, m + 2^k), both == m (mod 8) for k>=3, so
each core's subtree is self-contained: zero cross-core communication.
Core-local row index for p>=8 is (p>>3)+6; rows 0..6 hold the replicated
levels 0..2 (p<8), row 7 the core's level-3 seed node.

Per core the kernel:
  1. builds tree levels as fp32 matmuls in column layout [256, cols]
     (fp32r would be ~4x faster on PE but its tf32-like rounding fails the
     fp32 accuracy envelope over a 16-deep chain),
  2. converts to row-major via PE transpose (levels <=15) or a fused
     lhsT=V trick (level 16), DMA-writing rows into three DRAM tables
     (levels<=15 / level-16 b=0 / b=1) so gathers start as each completes,
  3. dma_gather's the distinct output rows (host-sorted, deduped indices,
     nq=4 SWDGE queues, multi-packet) and writes them out via the ACT
     HWDGE ring (kept separate from the build's SP ring so a gather-blocked
     out-DMA never stalls the build pipeline).
Host side only shards/sorts/dedups indices and reassembles the output.
"""

import numpy as np

DIM = 256
NCORES = 8
L_MAX = 16          # deepest tree level (positions < 2^(L_MAX+1))
SEG = 1024          # gather segment size (per dma_gather call)
SINGLE_PACKET = False
NPARTS = 6          # lo0(levels<=L-2), lo1(level L-1), 4 quarters of level L


def _nrows(l_max):
    return 7 + (1 << (l_max - 2)) - 1


# ---------------------------------------------------------------------------
# host-side sharding
# ---------------------------------------------------------------------------

def _seg_plan(maxcount, seg):
    sizes = []
    rem = max(maxcount, 128)
    while rem > seg:
        sizes.append(seg)
        rem -= seg
    sizes.append(max(128, -(-rem // 128) * 128))
    return sizes


def host_preprocess(unique, l_max=L_MAX, seg=SEG):
    """Shard positions by p mod 8, sort each core's local row indices.

    The per-core table is split into table_lo (levels <= l_max-1) and
    table_hi (the deepest level) so gathers from _lo never conflict with
    _hi writes.  Segments are planned separately per half."""
    u = np.asarray(unique).astype(np.int64)
    core = u & 7
    loc = np.where(u >= 8, (u >> 3) + 6, np.maximum(u - 1, 0))
    lv15_base = 6 + (1 << (l_max - 4))   # first row of level l_max-1
    lv16_base = 6 + (1 << (l_max - 3))   # first row of the deepest level
    hi_q = 1 << (l_max - 5)              # quarter of the deepest level
    bounds = [0, lv15_base, lv16_base] +         [lv16_base + k * hi_q for k in (1, 2, 3, 4)]

    halves = []   # per part: (percore list, seg_sizes)
    for part in range(NPARTS):
        percore = []
        for i in range(NCORES):
            sel = (core == i) & (loc >= bounds[part]) & (loc < bounds[part + 1])
            pos = np.nonzero(sel)[0]
            li = loc[pos] - bounds[part]
            # dedup: gather each distinct row once; host expands duplicates
            li_u = np.unique(li) if len(li) else li
            rank = np.searchsorted(li_u, li)
            percore.append((li_u, pos, rank))
        maxcount = max(len(li_u) for li_u, _, _ in percore)
        halves.append((percore, _seg_plan(maxcount, seg)))

    seg_sizes = []
    seg_src = []
    for part in range(NPARTS):
        seg_sizes += halves[part][1]
        seg_src += [part] * len(halves[part][1])
    nseg = len(seg_sizes)

    idxseg = np.zeros((NCORES, nseg, 128, seg // 16), np.int16)
    s0 = 0
    for half in range(NPARTS):
        percore, sizes = halves[half]
        starts = np.cumsum([0] + sizes)
        for si, ns in enumerate(sizes):
            s = s0 + si
            for i in range(NCORES):
                li, _, _ = percore[i]
                chunk = li[starts[si]:starts[si] + ns]
                buf = np.zeros(ns, np.int64)
                buf[:len(chunk)] = chunk
                w = buf.reshape(ns // 16, 16).T.astype(np.int16)
                idxseg[i, s, :, : ns // 16] = np.tile(w, (8, 1))
        s0 += len(sizes)

    return dict(
        halves=halves, seg_sizes=seg_sizes, seg_src=seg_src,
        idxseg=idxseg, nseg=nseg, seg=seg, lv16_base=lv16_base,
    )


def host_postprocess(results, pre, n_out, dtype=np.float32):
    """Scatter per-core gathered rows back into the full output."""
    seg_sizes, seg = pre["seg_sizes"], pre["seg"]
    out = np.zeros((n_out, DIM), dtype)
    nsegs = [len(pre["halves"][p][1]) for p in range(NPARTS)]
    for i in range(NCORES):
        arr = results[i]["out"].reshape(len(seg_sizes), 128, seg // 128, DIM)
        rows = []
        for s, ns in enumerate(seg_sizes):
            # gathered row j -> [partition j%128, slot j//128]
            rows.append(arr[s, :, : ns // 128].transpose(1, 0, 2).reshape(-1, DIM))
        s0 = 0
        for part in range(NPARTS):
            rws = np.concatenate(rows[s0:s0 + nsegs[part]], axis=0)
            s0 += nsegs[part]
            li_u, pos, rank = pre["halves"][part][0][i]
            if len(pos):
                out[pos] = rws[rank]
    return out


# ---------------------------------------------------------------------------
# device program
# ---------------------------------------------------------------------------

def build_program(seg_sizes, seg_src, l_max=L_MAX, seg=SEG, use_f32r=True,
                  nq=4):
    import concourse.bass as bass
    import concourse.tile as tile
    import concourse.mybir as mybir
    from concourse import bacc
    from concourse.masks import make_identity

    f32 = mybir.dt.float32
    f32r = mybir.dt.float32r
    i16 = mybir.dt.int16
    mdt = f32r if use_f32r else f32   # matmul input dtype
    MUL = mybir.AluOpType.mult
    AX_X = mybir.AxisListType.X

    nrows = _nrows(l_max)
    nseg = len(seg_sizes)

    nc = bacc.Bacc("TRN2", target_bir_lowering=False, debug=False,
                   num_devices=NCORES, num_swdge_queues=nq,
                   dynamic_dma_scratch_size=65536)

    primsT = nc.dram_tensor("primsT", [2, DIM, DIM], f32, kind="ExternalInput").ap()
    ident = nc.dram_tensor("identity", [1, DIM], f32, kind="ExternalInput").ap()
    selrep = nc.dram_tensor("selrep", [128, NCORES], f32, kind="ExternalInput").ap()
    idxseg = nc.dram_tensor("idxseg", [nseg, 128, seg // 16], i16,
                            kind="ExternalInput").ap()
    out = nc.dram_tensor("out", [nseg, 128, (seg // 128) * DIM], f32,
                         kind="ExternalOutput").ap()

    from contextlib import ExitStack
    with tile.TileContext(nc) as tc:
        with ExitStack() as ctx:
            cpool = ctx.enter_context(tc.tile_pool(name="consts", bufs=1))
            vpool = ctx.enter_context(tc.tile_pool(name="vbufs", bufs=1))
            stg_pool = ctx.enter_context(tc.tile_pool(name="stg", bufs=4))
            gpool = ctx.enter_context(tc.tile_pool(name="gath", bufs=3))
            ipool = ctx.enter_context(tc.tile_pool(name="idx", bufs=max(1, nseg)))
            pcols = ctx.enter_context(tc.tile_pool(name="pcols", bufs=6, space="PSUM"))
            prow = ctx.enter_context(tc.tile_pool(name="prow", bufs=2, space="PSUM"))
            dpool = ctx.enter_context(tc.tile_pool(name="dram", bufs=1, space="DRAM"))

            lv15_base = 6 + (1 << (l_max - 4))
            hi_q = 1 << (l_max - 5)
            table_lo0 = dpool.tile([lv15_base, DIM], f32, name="table_lo0")
            table_lo1 = dpool.tile([1 << (l_max - 4), DIM], f32, name="table_lo1")
            table_hiq = [dpool.tile([hi_q, DIM], f32, name=f"table_hiq{k}")
                         for k in range(4)]
            tables = (table_lo0, table_lo1) + tuple(table_hiq)

            # ---- gather index tiles: load first so the sync ring serves
            # them before the build's row-write DMA stream ------------------
            itiles = []
            for s in range(nseg):
                it = ipool.tile([128, seg // 16], i16, tag="it", name="it")
                nc.sync.dma_start(it[:], idxseg[s])
                itiles.append(it)

            # ---- constants -------------------------------------------------
            pT = [[None, None], [None, None]]
            for b in range(2):
                for j in range(2):
                    raw = cpool.tile([128, DIM], f32, tag=f"pTr{b}{j}",
                                     name=f"pTr{b}{j}")
                    nc.sync.dma_start(raw[:], primsT[b, 128 * j:128 * (j + 1), :])
                    t = cpool.tile([128, DIM], mdt, tag=f"pT{b}{j}", name=f"pT{b}{j}")
                    nc.vector.tensor_copy(t[:], raw[:])
                    pT[b][j] = t
            ptcat = []
            for j in range(2):
                t = cpool.tile([128, 2 * DIM], mdt, tag=f"ptcat{j}", name=f"ptcat{j}")
                for b in range(2):
                    nc.vector.tensor_copy(t[:, b * DIM:(b + 1) * DIM], pT[b][j][:])
                ptcat.append(t)
            identm_raw = cpool.tile([128, 128], f32, tag="identmr", name="identmr")
            make_identity(nc, identm_raw[:])
            identm = cpool.tile([128, 128], mdt, tag="identm", name="identm")
            nc.vector.tensor_copy(identm[:], identm_raw[:])
            selt = cpool.tile([128, NCORES], f32, tag="sel", name="selt")
            nc.sync.dma_start(selt[:], selrep[:, :])
            v0 = []
            ident_col = ident.rearrange("a (j p) -> j p a", p=128)
            for j in range(2):
                raw = cpool.tile([128, 1], f32, tag=f"v0r{j}", name=f"v0r{j}")
                nc.sync.dma_start(raw[:], ident_col[j])
                # width 2: fp32r matmuls need an even moving dim
                t = cpool.tile([128, 2], mdt, tag=f"v0{j}", name=f"v0{j}")
                nc.vector.tensor_copy(t[:], raw[:].to_broadcast([128, 2]))
                v0.append(t)

            # ---- helpers ---------------------------------------------------
            def psum_copy(dst_ap, src_ap):
                # DVE only: the ACT queue carries gather-side DMAs, which may
                # block on gather completion; copies must never sit behind them
                nc.vector.tensor_copy(dst_ap, src_ap)

            def build_children(V, c, parity):
                """V: [2][128, c] col-layout level; returns child col tiles."""
                cc = 2 * c
                Vn = [vpool.tile([128, max(cc, 1)], mdt, tag=f"V{j}p{parity}",
                                 name=f"Vn{j}")
                      for j in range(2)]
                for chunk in range(0, c, 512):
                    n = min(512, c - chunk)
                    npad = n + (n % 2)      # fp32r needs even moving dim
                    for b in range(2):
                        for i in range(2):
                            ps = pcols.tile([128, npad], f32, tag="pc", name="pc")
                            nc.tensor.matmul(
                                ps[:], pT[b][0][:, 128 * i:128 * (i + 1)],
                                V[0][:, chunk:chunk + npad],
                                start=True, stop=False)
                            nc.tensor.matmul(
                                ps[:], pT[b][1][:, 128 * i:128 * (i + 1)],
                                V[1][:, chunk:chunk + npad],
                                start=False, stop=True)
                            psum_copy(Vn[i][:, b * c + chunk: b * c + chunk + n],
                                      ps[:, :n])
                return Vn

            def emit_rows_small(V, c, row_base):
                """c <= 128 columns -> c table rows starting at row_base."""
                ps = prow.tile([128, 2 * 128], f32, tag="pr", name="pr")
                for j in range(2):
                    nc.tensor.transpose(ps[:c, 128 * j:128 * (j + 1)].bitcast(mdt),
                                        V[j][:, :c], identm[:])
                st = stg_pool.tile([128, 4 * DIM], f32, tag="st", name="st")
                psum_copy(st[:c, :DIM], ps[:c, :DIM])
                tab, rb = ((table_lo0, row_base) if row_base < lv15_base
                           else (table_lo1, row_base - lv15_base))
                nc.sync.dma_start(tab[rb:rb + c, :], st[:c, :DIM])

            def emit_rows_groups(V, c, row_base):
                """c > 128 columns: 128-col groups, batched 4 groups per DMA."""
                ngroups = c // 128
                for g0 in range(0, ngroups, 4):
                    nb = min(4, ngroups - g0)
                    st = stg_pool.tile([128, 4 * DIM], f32, tag="st", name="st")
                    for gg in range(nb):
                        g = g0 + gg
                        ps = prow.tile([128, 2 * 128], f32, tag="pr", name="pr")
                        for j in range(2):
                            nc.tensor.transpose(
                                ps[:, 128 * j:128 * (j + 1)].bitcast(mdt),
                                V[j][:, 128 * g:128 * (g + 1)], identm[:])
                        psum_copy(st[:, DIM * gg:DIM * (gg + 1)], ps[:, :DIM])
                    r0 = row_base + 128 * g0
                    tab, rb = ((table_lo0, r0) if row_base < lv15_base
                               else (table_lo1, r0 - lv15_base))
                    dst = tab[rb:rb + 128 * nb, :].rearrange(
                        "(g p) d -> p g d", p=128)
                    nc.sync.dma_start(dst, st[:, :DIM * nb])

            # ---- global levels 0..3, seed selection ------------------------
            emit_rows_small(v0, 1, 0)                      # row 0 (p=0,1)
            V, c = v0, 1
            rowptr = 1
            for lvl in range(1, 4):                        # child level lvl
                V = build_children(V, c, lvl % 2)
                c *= 2
                if lvl <= 2:
                    emit_rows_small(V, c, rowptr)          # rows 1..6
                    rowptr += c
            seeds = []
            for j in range(2):
                tmp = cpool.tile([128, NCORES], f32, tag=f"seedtmp{j}", name=f"seedtmp{j}")
                nc.vector.tensor_tensor(tmp[:], V[j][:, :NCORES].bitcast(f32),
                                        selt[:], op=MUL)
                sdr = cpool.tile([128, 1], f32, tag=f"seedr{j}", name=f"seedr{j}")
                nc.vector.reduce_sum(sdr[:], tmp[:], axis=AX_X)
                sd = cpool.tile([128, 2], mdt, tag=f"seed{j}", name=f"seed{j}")
                nc.vector.tensor_copy(sd[:], sdr[:].to_broadcast([128, 2]))
                seeds.append(sd)
            emit_rows_small(seeds, 1, 7)                   # row 7 (seed)

            # ---- per-core levels 4..L_MAX ----------------------------------
            V, c = seeds, 1
            for kk in range(3, l_max):                     # child level kk+1
                child_base = 6 + (1 << (kk - 2))
                if kk + 1 < l_max:
                    V = build_children(V, c, kk % 2)
                    c *= 2
                    if c <= 128:
                        emit_rows_small(V, c, child_base)
                    else:
                        emit_rows_groups(V, c, child_base)
                else:
                    # deepest level: rows for BOTH prims in one psum bank,
                    # (P_b @ V)^T = V^T @ P_b^T with rhs = [P0^T_j | P1^T_j]
                    ngroups = -(-c // 128)
                    gq = hi_q // 128      # groups per quarter
                    for g0 in range(0, ngroups, 4):
                        nb = min(4, ngroups - g0)
                        sts = []
                        for b in range(2):
                            sts.append(stg_pool.tile([128, 4 * DIM], f32,
                                                     tag="st", name=f"st16{b}"))
                        cgs = []
                        for gg in range(nb):
                            g = g0 + gg
                            cg = min(128, c - 128 * g)
                            cgs.append(cg)
                            ps = pcols.tile([128, 512], f32, tag="pc", name="pc16")
                            nc.tensor.matmul(
                                ps[:cg, :],
                                V[0][:, 128 * g:128 * g + cg],
                                ptcat[0][:],
                                start=True, stop=False)
                            nc.tensor.matmul(
                                ps[:cg, :],
                                V[1][:, 128 * g:128 * g + cg],
                                ptcat[1][:],
                                start=False, stop=True)
                            for b in range(2):
                                psum_copy(sts[b][:cg, DIM * gg:DIM * (gg + 1)],
                                          ps[:cg, b * DIM:(b + 1) * DIM])
                        for b in range(2):
                            if gq:
                                tab_b = table_hiq[b * 2 + min(1, g0 // gq)]
                                r0 = (128 * g0) % hi_q
                            else:
                                tab_b = table_hiq[b * 2]
                                r0 = 0
                            st = sts[b]
                            if nb == 1 and cgs[0] < 128:
                                nc.sync.dma_start(
                                    tab_b[r0:r0 + cgs[0], :], st[:cgs[0], :DIM])
                            else:
                                dst = tab_b[r0:r0 + 128 * nb, :].rearrange(
                                    "(g p) d -> p g d", p=128)
                                nc.sync.dma_start(dst, st[:, :DIM * nb])

            # ---- gather + output -------------------------------------------
            for s, ns in enumerate(seg_sizes):
                it = itiles[s]
                gt = gpool.tile([128, seg // 128, DIM], f32, tag="gt", name="gt")
                src_t = tables[seg_src[s]]
                nc.gpsimd.dma_gather(
                    gt[:, : ns // 128, :],
                    src_t[:, :],
                    it[:, : ns // 16],
                    ns, ns, DIM, queue_num=s % nq,
                    single_packet=SINGLE_PACKET)
                nc.scalar.dma_start(out[s, :, : (ns // 128) * DIM],
                                    gt[:, : ns // 128, :])

    nc.compile()
    return nc


# ---------------------------------------------------------------------------
# entry point
# ---------------------------------------------------------------------------

_PROGRAM_CACHE = {}


def _run(unique, primitives, identity, l_max=L_MAX, seg=SEG, use_f32r=False,
         nq=4, **run_kwargs):
    from concourse.bass_utils import run_bass_kernel_spmd

    unique = np.asarray(unique)
    primitives = np.ascontiguousarray(np.asarray(primitives, np.float32))
    identity = np.ascontiguousarray(np.asarray(identity, np.float32))

    pre = host_preprocess(unique, l_max=l_max, seg=seg)
    key = (l_max, seg, use_f32r, nq,
           tuple(pre["seg_sizes"]), tuple(pre["seg_src"]))
    if key not in _PROGRAM_CACHE:
        _PROGRAM_CACHE[key] = build_program(pre["seg_sizes"], pre["seg_src"],
                                            l_max=l_max, seg=seg,
                                            use_f32r=use_f32r, nq=nq)
    nc = _PROGRAM_CACHE[key]

    primsT = np.ascontiguousarray(primitives.transpose(0, 2, 1))
    in_maps = []
    for i in range(NCORES):
        sel = np.zeros((128, NCORES), np.float32)
        sel[:, i] = 1.0
        in_maps.append({
            "primsT": primsT,
            "identity": identity,
            "selrep": sel,
            "idxseg": np.ascontiguousarray(pre["idxseg"][i]),
        })

    res = run_bass_kernel_spmd(nc, in_maps, core_ids=list(range(NCORES)),
                               **run_kwargs)
    out = host_postprocess(res.results, pre, len(unique))
    return out, res


def kernel(unique, primitives, identity):
    out, _ = _run(unique, primitives, identity)
    return out


if __name__ == "__main__":
    # tiny smoke run (full shapes) — prefer test.py for the real check
    rng = np.random.default_rng(0)
    u = rng.integers(0, 1 << 17, size=131072).astype(np.int32)
    prims = rng.standard_normal((2, DIM, DIM)).astype(np.float32)
    ones = np.ones((1, DIM), np.float32)
    out = kernel(u, prims, ones)
    print("kernel output", out.shape, out.dtype)



# revision 6
# speedup vs baseline: 2.3217x; 2.3217x over previous
"""Trainium2 Bass kernel for nn_BinaryPathEncoder (v2 — tree table, no gather).

Math: output row for position p is ones(256) pushed through a chain of
matrices P0/P1 chosen by the bits of p (LSB-first, topmost set bit dropped).
All distinct bit-paths form a complete binary tree with 2^17-1 nodes; level
k+1 of the tree is [P0 @ V_k, P1 @ V_k] so the whole tree costs ~17 GFLOP.
Every output row is then a lookup into the tree table.

Sharding: tree nodes (k>=3, m) are assigned to core p mod 8; children of a
core's node stay on that core, so each core's subtree is self-contained with
zero cross-core communication.  The host computes the tiny levels 0..9
(1023 rows, 0.8% of the table, 0.13 GFLOP in numpy) and hands each core its
64 level-9 seed vectors; the device builds levels 10..16 (16256 rows/core,
99% of the FLOPs) as fp32r matmuls in column layout [dim, nodes]:

  for k in 10..16:  child half (b,i) += P_b^T[j-half, i-half]^T @ V[j-half]

Each level is emitted to DRAM as bf16 column tiles (one bf16 rounding at
the end of the chain; fp32r keeps the chain itself at ~1e-3 rel err, well
inside the 2e-2 envelope).  No on-device gather: the full per-core table
streams out (8.3 MB/core) and the host does the final index lookup, which
it already needed for scatter/dedup anyway.  Copy work is spread over three
engines (DVE/ACT alternate on PSUM drains, Pool does the SBUF f32->bf16
emit conversion) so the PE build (~29 us) stays the critical path.
"""

import numpy as np
import ml_dtypes

DIM = 256
NCORES = 8
DEV_LV0 = 10                       # first device-built level
L_MAX = 16                         # deepest tree level (positions < 2^17)
SEED_C = 1 << (DEV_LV0 - 4)        # 64 level-(DEV_LV0-1) cols per core
OUT_COLS = (1 << (L_MAX - 2)) - 2 * SEED_C   # 16256 cols, levels 10..16
VMAX = 1 << (L_MAX - 4)            # widest chain level (level 15: 4096)


# ---------------------------------------------------------------------------
# device program
# ---------------------------------------------------------------------------

def build_program():
    import concourse.tile as tile
    import concourse.mybir as mybir
    from concourse import bacc

    f32 = mybir.dt.float32
    f32r = mybir.dt.float32r
    bf16 = mybir.dt.bfloat16
    COPY = mybir.ActivationFunctionType.Copy

    nc = bacc.Bacc("TRN2", target_bir_lowering=False, debug=False,
                   num_devices=NCORES)

    primsT = nc.dram_tensor("primsT", [2, DIM, DIM], f32,
                            kind="ExternalInput").ap()
    seeds = nc.dram_tensor("seeds", [2, 128, SEED_C], f32,
                           kind="ExternalInput").ap()
    out = nc.dram_tensor("out", [2, 128, OUT_COLS], bf16,
                         kind="ExternalOutput").ap()

    from contextlib import ExitStack
    with tile.TileContext(nc) as tc:
        with ExitStack() as ctx:
            cpool = ctx.enter_context(tc.tile_pool(name="consts", bufs=1))
            vpool = ctx.enter_context(tc.tile_pool(name="vbufs", bufs=2))
            epool = ctx.enter_context(tc.tile_pool(name="emit", bufs=2))
            hpool = ctx.enter_context(tc.tile_pool(name="emith", bufs=1))
            ppool = ctx.enter_context(tc.tile_pool(name="pc", bufs=2,
                                                   space="PSUM"))

            # ---- constants -----------------------------------------------
            # fp32r operands must be produced as f32r (the writing engine
            # applies the rounding) — DMA the raw f32 then convert-copy.
            pT = [[None, None], [None, None]]
            for b in range(2):
                for j in range(2):
                    raw = cpool.tile([128, DIM], f32, tag=f"pTr{b}{j}",
                                     name=f"pTr{b}{j}")
                    nc.sync.dma_start(raw[:],
                                      primsT[b, 128 * j:128 * (j + 1), :])
                    t = cpool.tile([128, DIM], f32r, tag=f"pT{b}{j}",
                                   name=f"pT{b}{j}")
                    nc.vector.tensor_copy(t[:], raw[:])
                    pT[b][j] = t
            V = []
            for j in range(2):
                raw = cpool.tile([128, SEED_C], f32, tag=f"V9r{j}",
                                 name=f"V9r{j}")
                nc.sync.dma_start(raw[:], seeds[j])
                t = cpool.tile([128, SEED_C], f32r, tag=f"V9{j}",
                               name=f"V9{j}")
                nc.vector.tensor_copy(t[:], raw[:])
                V.append(t)

            # ---- levels DEV_LV0..L_MAX -----------------------------------
            c = SEED_C
            off = 0
            for k in range(DEV_LV0, L_MAX + 1):
                cc = 2 * c
                last = k == L_MAX
                if not last:
                    newV = [vpool.tile([128, VMAX], f32r, tag=f"V{j}",
                                       name=f"V{k}_{j}")[:, :cc]
                            for j in range(2)]
                    emit = [epool.tile([128, VMAX], bf16, tag=f"E{j}",
                                       name=f"E{k}_{j}")[:, :cc]
                            for j in range(2)]
                else:
                    emit = [hpool.tile([128, cc], bf16, tag=f"H{j}",
                                       name=f"H{j}")
                            for j in range(2)]
                for b in range(2):
                    for i in range(2):
                        wslice = slice(128 * i, 128 * (i + 1))
                        for s0 in range(0, c, 2048):
                            w = min(2048, c - s0)
                            ps = ppool.tile([128, 2048], f32, tag="PC",
                                            name="ps")[:, :w]
                            for q0 in range(0, w, 512):
                                qw = min(512, w - q0)
                                nc.tensor.matmul(
                                    ps[:, q0:q0 + qw],
                                    pT[b][0][:, wslice],
                                    V[0][:, s0 + q0:s0 + q0 + qw],
                                    start=True, stop=False)
                                nc.tensor.matmul(
                                    ps[:, q0:q0 + qw],
                                    pT[b][1][:, wslice],
                                    V[1][:, s0 + q0:s0 + q0 + qw],
                                    start=False, stop=True)
                            d0 = b * c + s0
                            if last:
                                # bf16 emit of the deepest level -> ACT
                                nc.scalar.activation(emit[i][:, d0:d0 + w],
                                                     ps[:, :w], COPY)
                            else:
                                # f32r chain drain (rounding) -> DVE
                                nc.vector.tensor_copy(newV[i][:, d0:d0 + w],
                                                      ps[:, :w])
                        if last:
                            # b-block of half i complete -> stream out
                            nc.sync.dma_start(
                                out[i, :, off + b * c:off + b * c + c],
                                emit[i][:, b * c:b * c + c])
                if not last:
                    for i in range(2):
                        nc.gpsimd.tensor_copy(emit[i][:, :cc],
                                              newV[i][:, :cc].bitcast(f32))
                        nc.sync.dma_start(out[i, :, off:off + cc],
                                          emit[i][:, :cc])
                    V = newV
                c = cc
                off += cc

    nc.compile()
    return nc


# ---------------------------------------------------------------------------
# host side
# ---------------------------------------------------------------------------

def _host_levels(primitives, identity):
    """Table rows for p < 2^DEV_LV0 (levels 0..DEV_LV0-1) in fp32."""
    T = np.zeros((1 << DEV_LV0, DIM), np.float32)
    T[0] = identity[0]
    T[1] = identity[0]
    for k in range(1, DEV_LV0):
        prev = T[1 << (k - 1):1 << k]
        half = 1 << (k - 1)
        T[1 << k:(1 << k) + half] = prev @ primitives[0].T
        T[(1 << k) + half:1 << (k + 1)] = prev @ primitives[1].T
    return T


_PROGRAM_CACHE = {}


def _run(unique, primitives, identity, **run_kwargs):
    from concourse.bass_utils import run_bass_kernel_spmd

    unique = np.asarray(unique)
    primitives = np.ascontiguousarray(np.asarray(primitives, np.float32))
    identity = np.ascontiguousarray(np.asarray(identity, np.float32))

    if "prog" not in _PROGRAM_CACHE:
        _PROGRAM_CACHE["prog"] = build_program()
    nc = _PROGRAM_CACHE["prog"]

    Th = _host_levels(primitives, identity)          # rows p < 1024
    primsT = np.ascontiguousarray(primitives.transpose(0, 2, 1))

    in_maps = []
    for core in range(NCORES):
        sc = Th[(1 << (DEV_LV0 - 1)) + core:1 << DEV_LV0:NCORES]  # [64, 256]
        seeds = np.ascontiguousarray(sc.T.reshape(2, 128, SEED_C))
        in_maps.append({"primsT": primsT, "seeds": seeds})

    res = run_bass_kernel_spmd(nc, in_maps, core_ids=list(range(NCORES)),
                               **run_kwargs)

    # assemble the full table, then one bulk lookup
    Tfull = np.zeros((1 << (L_MAX + 1), DIM), np.float32)
    Tfull[:1 << DEV_LV0] = Th
    for core in range(NCORES):
        lo = np.asarray(res.results[core]["out"])
        lo = lo.view(ml_dtypes.bfloat16) if lo.dtype != ml_dtypes.bfloat16 \
            else lo
        lo = lo.astype(np.float32).reshape(2 * 128, OUT_COLS)
        off = 0
        for k in range(DEV_LV0, L_MAX + 1):
            cc = 1 << (k - 3)
            Tfull[(1 << k) + core:1 << (k + 1):NCORES] = lo[:, off:off + cc].T
            off += cc
    outv = Tfull[unique.astype(np.int64)]
    return outv, res


def kernel(unique, primitives, identity):
    out, _ = _run(unique, primitives, identity)
    return out


if __name__ == "__main__":
    # tiny smoke run (full shapes) — prefer test.py for the real check
    rng = np.random.default_rng(0)
    u = rng.integers(0, 1 << 17, size=131072).astype(np.int32)
    prims = rng.standard_normal((2, DIM, DIM)).astype(np.float32)
    ones = np.ones((1, DIM), np.float32)
    out = kernel(u, prims, ones)
    print("kernel output", out.shape, out.dtype)


# revision 7
# speedup vs baseline: 3.1218x; 1.3446x over previous
"""Trainium2 Bass kernel for nn_BinaryPathEncoder (v3 — tree table, no gather).

Math: output row for position p is ones(256) pushed through a chain of
matrices P0/P1 chosen by the bits of p (LSB-first, topmost set bit dropped).
All distinct bit-paths form a complete binary tree with 2^17-1 nodes; level
k+1 of the tree is [P0 @ V_k, P1 @ V_k] so the whole tree costs ~17 GFLOP.
Every output row is then a lookup into the tree table.

Sharding: tree nodes are assigned to core p mod 8; children of a core's
node stay on that core, so each core's subtree is self-contained with zero
cross-core communication.  The host computes the tiny levels 0..9 (1023
rows, 0.8% of the table) and hands each core its 64 level-9 seed vectors;
the device builds levels 10..16 (16256 rows/core, 99% of the FLOPs) as
fp32r matmuls in column layout [dim, nodes]:

  child half (b,i) = sum_j P_b^T[128j:, 128i:]^T @ V[j]     (PSUM f32)

fp32r runs the PE at bf16 rate for moving dims >= 256 and keeps the chain
at ~1e-3 rel err (vs the 2e-2 envelope); a single bf16 rounding happens
only at emission.  No on-device gather: the full per-core table streams
out (~11.5 MB/core) and the host does the final index lookup.

Emission paths (so no single engine trails the PE):
  levels 10..13 -> DVE bf16 copy of the chain tile, SP HWDGE ring
  levels 14..15 -> raw f32 DMA of the chain tile itself, ACT HWDGE ring
  level 16      -> ACT bf16 drains straight from PSUM, SP HWDGE ring
Chain drains (PSUM -> f32r V tiles) alternate DVE/ACT per phase because a
lone DVE (0.96 GHz) cannot match the PE's 1.2 GHz-equivalent column rate.
"""

import numpy as np
import ml_dtypes

DIM = 256
NCORES = 8
DEV_LV0 = 10                       # first device-built level
L_MAX = 16                         # deepest tree level (positions < 2^17)
SEED_C = 1 << (DEV_LV0 - 4)        # 64 level-(DEV_LV0-1) cols per core
VMAX = 1 << (L_MAX - 4)            # widest chain level (level 15: 4096)
BF_COLS = 128 + 256 + 512 + 1024 + 8192   # levels 10..13 + 16
F32_COLS = 2048 + 4096                    # levels 14..15


# ---------------------------------------------------------------------------
# device program
# ---------------------------------------------------------------------------

def build_program():
    import concourse.tile as tile
    import concourse.mybir as mybir
    from concourse import bacc

    f32 = mybir.dt.float32
    f32r = mybir.dt.float32r
    bf16 = mybir.dt.bfloat16
    COPY = mybir.ActivationFunctionType.Copy

    nc = bacc.Bacc("TRN2", target_bir_lowering=False, debug=False,
                   num_devices=NCORES)

    primsT = nc.dram_tensor("primsT", [2, DIM, DIM], f32,
                            kind="ExternalInput").ap()
    seeds = nc.dram_tensor("seeds", [2, 128, SEED_C], f32,
                           kind="ExternalInput").ap()
    out_bf = nc.dram_tensor("out_bf", [2, 128, BF_COLS], bf16,
                            kind="ExternalOutput").ap()
    out_f32 = nc.dram_tensor("out_f32", [2, 128, F32_COLS], f32,
                             kind="ExternalOutput").ap()

    from contextlib import ExitStack
    with tile.TileContext(nc) as tc:
        with ExitStack() as ctx:
            cpool = ctx.enter_context(tc.tile_pool(name="consts", bufs=1))
            vpool = ctx.enter_context(tc.tile_pool(name="vbufs", bufs=2))
            epool = ctx.enter_context(tc.tile_pool(name="emit", bufs=2))
            hpool = ctx.enter_context(tc.tile_pool(name="emith", bufs=1))
            ppool = ctx.enter_context(tc.tile_pool(name="pc", bufs=2,
                                                   space="PSUM"))

            # ---- constants -----------------------------------------------
            # fp32r operands must be produced as f32r (the writing engine
            # applies the rounding) — DMA the raw f32 then convert-copy.
            pT = [[None, None], [None, None]]
            for b in range(2):
                for j in range(2):
                    raw = cpool.tile([128, DIM], f32, tag=f"pTr{b}{j}",
                                     name=f"pTr{b}{j}")
                    nc.sync.dma_start(raw[:],
                                      primsT[b, 128 * j:128 * (j + 1), :])
                    t = cpool.tile([128, DIM], f32r, tag=f"pT{b}{j}",
                                   name=f"pT{b}{j}")
                    nc.vector.tensor_copy(t[:], raw[:])
                    pT[b][j] = t
            V = []
            for j in range(2):
                raw = cpool.tile([128, SEED_C], f32, tag=f"V9r{j}",
                                 name=f"V9r{j}")
                nc.sync.dma_start(raw[:], seeds[j])
                t = cpool.tile([128, SEED_C], f32r, tag=f"V9{j}",
                               name=f"V9{j}")
                nc.vector.tensor_copy(t[:], raw[:])
                V.append(t)

            # chain drains alternate DVE / ACT so neither trails the PE
            flip = [0]

            def chain_drain(dst_ap, src_ap):
                if flip[0] % 2 == 0:
                    nc.vector.tensor_copy(dst_ap, src_ap)
                else:
                    nc.scalar.activation(dst_ap, src_ap, COPY)
                flip[0] += 1

            # ---- levels DEV_LV0..L_MAX -----------------------------------
            c = SEED_C
            off_bf = 0
            off_f = 0
            for k in range(DEV_LV0, L_MAX + 1):
                cc = 2 * c
                last = k == L_MAX
                small = k <= 13
                if not last:
                    newV = [vpool.tile([128, VMAX], f32r, tag=f"V{j}",
                                       name=f"V{k}_{j}")[:, :cc]
                            for j in range(2)]
                if small:
                    emit = [epool.tile([128, 1024], bf16, tag=f"E{j}",
                                       name=f"E{k}_{j}")[:, :cc]
                            for j in range(2)]
                elif last:
                    emit = [hpool.tile([128, cc], bf16, tag=f"H{j}",
                                       name=f"H{j}")
                            for j in range(2)]
                for b in range(2):
                    for i in range(2):
                        wslice = slice(128 * i, 128 * (i + 1))
                        for s0 in range(0, c, 2048):
                            w = min(2048, c - s0)
                            ps = ppool.tile([128, 2048], f32, tag="PC",
                                            name="ps")[:, :w]
                            for q0 in range(0, w, 512):
                                qw = min(512, w - q0)
                                nc.tensor.matmul(
                                    ps[:, q0:q0 + qw],
                                    pT[b][0][:, wslice],
                                    V[0][:, s0 + q0:s0 + q0 + qw],
                                    start=True, stop=False)
                                nc.tensor.matmul(
                                    ps[:, q0:q0 + qw],
                                    pT[b][1][:, wslice],
                                    V[1][:, s0 + q0:s0 + q0 + qw],
                                    start=False, stop=True)
                            d0 = b * c + s0
                            if last:
                                nc.scalar.activation(emit[i][:, d0:d0 + w],
                                                     ps[:, :w], COPY)
                            else:
                                chain_drain(newV[i][:, d0:d0 + w],
                                            ps[:, :w])
                        if last:
                            # b-block of half i complete -> stream out
                            nc.sync.dma_start(
                                out_bf[i, :, off_bf + b * c:
                                       off_bf + b * c + c],
                                emit[i][:, b * c:b * c + c])
                if small:
                    for i in range(2):
                        nc.vector.tensor_copy(emit[i][:, :cc],
                                              newV[i][:, :cc].bitcast(f32))
                        nc.sync.dma_start(out_bf[i, :, off_bf:off_bf + cc],
                                          emit[i][:, :cc])
                    off_bf += cc
                elif not last:
                    for i in range(2):
                        nc.scalar.dma_start(out_f32[i, :, off_f:off_f + cc],
                                            newV[i][:, :cc].bitcast(f32))
                    off_f += cc
                else:
                    off_bf += cc
                if not last:
                    V = newV
                c = cc

    nc.compile()
    return nc


# ---------------------------------------------------------------------------
# host side
# ---------------------------------------------------------------------------

def _host_levels(primitives, identity):
    """Table rows for p < 2^DEV_LV0 (levels 0..DEV_LV0-1) in fp32."""
    T = np.zeros((1 << DEV_LV0, DIM), np.float32)
    T[0] = identity[0]
    T[1] = identity[0]
    for k in range(1, DEV_LV0):
        prev = T[1 << (k - 1):1 << k]
        half = 1 << (k - 1)
        T[1 << k:(1 << k) + half] = prev @ primitives[0].T
        T[(1 << k) + half:1 << (k + 1)] = prev @ primitives[1].T
    return T


_PROGRAM_CACHE = {}


def _run(unique, primitives, identity, **run_kwargs):
    from concourse.bass_utils import run_bass_kernel_spmd

    unique = np.asarray(unique)
    primitives = np.ascontiguousarray(np.asarray(primitives, np.float32))
    identity = np.ascontiguousarray(np.asarray(identity, np.float32))

    if "prog" not in _PROGRAM_CACHE:
        _PROGRAM_CACHE["prog"] = build_program()
    nc = _PROGRAM_CACHE["prog"]

    Th = _host_levels(primitives, identity)          # rows p < 1024
    primsT = np.ascontiguousarray(primitives.transpose(0, 2, 1))

    in_maps = []
    for core in range(NCORES):
        sc = Th[(1 << (DEV_LV0 - 1)) + core:1 << DEV_LV0:NCORES]  # [64, 256]
        seeds = np.ascontiguousarray(sc.T.reshape(2, 128, SEED_C))
        in_maps.append({"primsT": primsT, "seeds": seeds})

    res = run_bass_kernel_spmd(nc, in_maps, core_ids=list(range(NCORES)),
                               **run_kwargs)

    # assemble the full table, then one bulk lookup
    Tfull = np.zeros((1 << (L_MAX + 1), DIM), np.float32)
    Tfull[:1 << DEV_LV0] = Th
    for core in range(NCORES):
        r = res.results[core]
        bf = np.asarray(r["out_bf"])
        if bf.dtype != ml_dtypes.bfloat16:
            bf = bf.view(ml_dtypes.bfloat16)
        bf = bf.astype(np.float32).reshape(2 * 128, BF_COLS)
        f3 = np.asarray(r["out_f32"]).reshape(2 * 128, F32_COLS)
        off_bf = 0
        off_f = 0
        for k in range(DEV_LV0, L_MAX + 1):
            cc = 1 << (k - 3)
            if k <= 13 or k == L_MAX:
                vals = bf[:, off_bf:off_bf + cc].T
                off_bf += cc
            else:
                vals = f3[:, off_f:off_f + cc].T
                off_f += cc
            Tfull[(1 << k) + core:1 << (k + 1):NCORES] = vals
    outv = Tfull[unique.astype(np.int64)]
    return outv, res


def kernel(unique, primitives, identity):
    out, _ = _run(unique, primitives, identity)
    return out


if __name__ == "__main__":
    # tiny smoke run (full shapes) — prefer test.py for the real check
    rng = np.random.default_rng(0)
    u = rng.integers(0, 1 << 17, size=131072).astype(np.int32)
    prims = rng.standard_normal((2, DIM, DIM)).astype(np.float32)
    ones = np.ones((1, DIM), np.float32)
    out = kernel(u, prims, ones)
    print("kernel output", out.shape, out.dtype)


# revision 11
# speedup vs baseline: 4.0160x; 1.2865x over previous
"""Trainium2 Bass kernel for nn_BinaryPathEncoder (v3 — tree table, no gather).

Math: output row for position p is ones(256) pushed through a chain of
matrices P0/P1 chosen by the bits of p (LSB-first, topmost set bit dropped).
All distinct bit-paths form a complete binary tree with 2^17-1 nodes; level
k+1 of the tree is [P0 @ V_k, P1 @ V_k] so the whole tree costs ~17 GFLOP.
Every output row is then a lookup into the tree table.

Sharding: tree nodes are assigned to core p mod 8; children of a core's
node stay on that core, so each core's subtree is self-contained with zero
cross-core communication.  The host computes the tiny levels 0..9 (1023
rows, 0.8% of the table) and hands each core its 64 level-9 seed vectors;
the device builds levels 10..16 (16256 rows/core, 99% of the FLOPs) as
fp32r matmuls in column layout [dim, nodes]:

  child half (b,i) = sum_j P_b^T[128j:, 128i:]^T @ V[j]     (PSUM f32)

fp32r runs the PE at bf16 rate for moving dims >= 256 and keeps the chain
at ~1e-3 rel err (vs the 2e-2 envelope); a single bf16 rounding happens
only at emission.  No on-device gather: the full per-core table streams
out (~11.5 MB/core) and the host does the final index lookup.

Emission paths (so no single engine trails the PE):
  levels 10..13 -> DVE bf16 copy of the chain tile, SP HWDGE ring
  levels 14..15 -> raw f32 DMA of the chain tile itself, ACT HWDGE ring
  level 16      -> ACT bf16 drains straight from PSUM, SP HWDGE ring
Chain drains (PSUM -> f32r V tiles) alternate DVE/ACT per phase because a
lone DVE (0.96 GHz) cannot match the PE's 1.2 GHz-equivalent column rate.
"""

import numpy as np
import ml_dtypes

DIM = 256
NCORES = 8
DEV_LV0 = 10                       # first device-built level
L_MAX = 16                         # deepest tree level (positions < 2^17)
SEED_C = 1 << (DEV_LV0 - 4)        # 64 level-(DEV_LV0-1) cols per core
VMAX = 1 << (L_MAX - 4)            # widest chain level (level 15: 4096)
BF_COLS = 128 + 256 + 512 + 1024 + 8192   # levels 10..13 + 16
F32_COLS = 2048 + 4096                    # levels 14..15


# ---------------------------------------------------------------------------
# device program
# ---------------------------------------------------------------------------

def build_program():
    import concourse.tile as tile
    import concourse.mybir as mybir
    from concourse import bacc

    f32 = mybir.dt.float32
    f32r = mybir.dt.float32r
    bf16 = mybir.dt.bfloat16
    COPY = mybir.ActivationFunctionType.Copy

    nc = bacc.Bacc("TRN2", target_bir_lowering=False, debug=False,
                   num_devices=NCORES)

    primsT = nc.dram_tensor("primsT", [2, DIM, DIM], f32,
                            kind="ExternalInput").ap()
    seeds = nc.dram_tensor("seeds", [2, 128, SEED_C], f32,
                           kind="ExternalInput").ap()
    out_bf = nc.dram_tensor("out_bf", [2, 128, BF_COLS], bf16,
                            kind="ExternalOutput").ap()
    out_f32 = nc.dram_tensor("out_f32", [2, 128, F32_COLS], f32,
                             kind="ExternalOutput").ap()

    from contextlib import ExitStack
    with tile.TileContext(nc) as tc:
        with ExitStack() as ctx:
            cpool = ctx.enter_context(tc.tile_pool(name="consts", bufs=1))
            vpool = ctx.enter_context(tc.tile_pool(name="vbufs", bufs=2))
            epool = ctx.enter_context(tc.tile_pool(name="emit", bufs=2))
            hpool = ctx.enter_context(tc.tile_pool(name="emith", bufs=1))
            ppool = ctx.enter_context(tc.tile_pool(name="pc", bufs=4,
                                                   space="PSUM"))

            # ---- constants -----------------------------------------------
            # fp32r operands must be produced as f32r (the writing engine
            # applies the rounding).  One batched DMA per input, split
            # across the two HWDGE rings, then one convert-copy each.
            sraw = cpool.tile([128, 2 * SEED_C], f32, tag="sraw", name="sraw")
            nc.sync.dma_start(sraw[:].rearrange("p (j m) -> p j m", j=2),
                              seeds.rearrange("j p m -> p j m"))
            praw = cpool.tile([128, 4 * DIM], f32, tag="praw", name="praw")
            nc.scalar.dma_start(praw[:].rearrange("p (g d) -> p g d", g=4),
                                primsT.rearrange("b (j p) d -> p (b j) d",
                                                 p=128))
            vs = cpool.tile([128, 2 * SEED_C], f32r, tag="vs", name="vs")
            nc.scalar.activation(vs[:], sraw[:], COPY)
            pt = cpool.tile([128, 4 * DIM], f32r, tag="pt", name="pt")
            nc.vector.tensor_copy(pt[:], praw[:])
            pT = [[pt[:, (2 * b + j) * DIM:(2 * b + j + 1) * DIM]
                   for j in range(2)] for b in range(2)]
            V = [vs[:, SEED_C * j:SEED_C * (j + 1)] for j in range(2)]

            # chain drains alternate DVE / ACT so neither trails the PE
            flip = [0]

            def chain_drain(dst_ap, src_ap):
                if flip[0] % 2 == 0:
                    nc.vector.tensor_copy(dst_ap, src_ap)
                else:
                    nc.scalar.activation(dst_ap, src_ap, COPY)
                flip[0] += 1

            # ---- levels DEV_LV0..L_MAX -----------------------------------
            c = SEED_C
            off_bf = 0
            off_f = 0
            for k in range(DEV_LV0, L_MAX + 1):
                cc = 2 * c
                last = k == L_MAX
                small = k <= 13
                if not last:
                    newV = [vpool.tile([128, VMAX], f32r, tag=f"V{j}",
                                       name=f"V{k}_{j}")[:, :cc]
                            for j in range(2)]
                if small:
                    emit = [epool.tile([128, 1024], bf16, tag=f"E{j}",
                                       name=f"E{k}_{j}")[:, :cc]
                            for j in range(2)]
                elif last:
                    emit = [hpool.tile([128, cc], bf16, tag=f"H{j}",
                                       name=f"H{j}")
                            for j in range(2)]
                for b in range(2):
                    for i in range(2):
                        wslice = slice(128 * i, 128 * (i + 1))
                        for s0 in range(0, c, 1024):
                            w = min(1024, c - s0)
                            ps = ppool.tile([128, 1024], f32, tag="PC",
                                            name="ps")[:, :w]
                            for q0 in range(0, w, 512):
                                qw = min(512, w - q0)
                                nc.tensor.matmul(
                                    ps[:, q0:q0 + qw],
                                    pT[b][0][:, wslice],
                                    V[0][:, s0 + q0:s0 + q0 + qw],
                                    start=True, stop=False)
                                nc.tensor.matmul(
                                    ps[:, q0:q0 + qw],
                                    pT[b][1][:, wslice],
                                    V[1][:, s0 + q0:s0 + q0 + qw],
                                    start=False, stop=True)
                            d0 = b * c + s0
                            chain_drain((emit[i] if last else newV[i])
                                        [:, d0:d0 + w], ps[:, :w])
                            if last and (s0 + w) % 2048 == 0:
                                # flush each 2048-col chunk as it completes
                                f0 = b * c + s0 + w - 2048
                                nc.sync.dma_start(
                                    out_bf[i, :, off_bf + f0:
                                           off_bf + f0 + 2048],
                                    emit[i][:, f0:f0 + 2048])
                if small:
                    for i in range(2):
                        nc.vector.tensor_copy(emit[i][:, :cc],
                                              newV[i][:, :cc].bitcast(f32))
                        nc.sync.dma_start(out_bf[i, :, off_bf:off_bf + cc],
                                          emit[i][:, :cc])
                    off_bf += cc
                elif not last:
                    for i in range(2):
                        nc.scalar.dma_start(out_f32[i, :, off_f:off_f + cc],
                                            newV[i][:, :cc].bitcast(f32))
                    off_f += cc
                else:
                    off_bf += cc
                if not last:
                    V = newV
                c = cc

    nc.compile()
    return nc


# ---------------------------------------------------------------------------
# host side
# ---------------------------------------------------------------------------

def _host_levels(primitives, identity):
    """Table rows for p < 2^DEV_LV0 (levels 0..DEV_LV0-1) in fp32."""
    T = np.zeros((1 << DEV_LV0, DIM), np.float32)
    T[0] = identity[0]
    T[1] = identity[0]
    for k in range(1, DEV_LV0):
        prev = T[1 << (k - 1):1 << k]
        half = 1 << (k - 1)
        T[1 << k:(1 << k) + half] = prev @ primitives[0].T
        T[(1 << k) + half:1 << (k + 1)] = prev @ primitives[1].T
    return T


_PROGRAM_CACHE = {}


def _run(unique, primitives, identity, **run_kwargs):
    from concourse.bass_utils import run_bass_kernel_spmd

    unique = np.asarray(unique)
    primitives = np.ascontiguousarray(np.asarray(primitives, np.float32))
    identity = np.ascontiguousarray(np.asarray(identity, np.float32))

    if "prog" not in _PROGRAM_CACHE:
        _PROGRAM_CACHE["prog"] = build_program()
    nc = _PROGRAM_CACHE["prog"]

    Th = _host_levels(primitives, identity)          # rows p < 1024
    primsT = np.ascontiguousarray(primitives.transpose(0, 2, 1))

    in_maps = []
    for core in range(NCORES):
        sc = Th[(1 << (DEV_LV0 - 1)) + core:1 << DEV_LV0:NCORES]  # [64, 256]
        seeds = np.ascontiguousarray(sc.T.reshape(2, 128, SEED_C))
        in_maps.append({"primsT": primsT, "seeds": seeds})

    res = run_bass_kernel_spmd(nc, in_maps, core_ids=list(range(NCORES)),
                               **run_kwargs)

    # assemble the full table, then one bulk lookup
    Tfull = np.zeros((1 << (L_MAX + 1), DIM), np.float32)
    Tfull[:1 << DEV_LV0] = Th
    for core in range(NCORES):
        r = res.results[core]
        bf = np.asarray(r["out_bf"])
        if bf.dtype != ml_dtypes.bfloat16:
            bf = bf.view(ml_dtypes.bfloat16)
        bf = bf.astype(np.float32).reshape(2 * 128, BF_COLS)
        f3 = np.asarray(r["out_f32"]).reshape(2 * 128, F32_COLS)
        off_bf = 0
        off_f = 0
        for k in range(DEV_LV0, L_MAX + 1):
            cc = 1 << (k - 3)
            if k <= 13 or k == L_MAX:
                vals = bf[:, off_bf:off_bf + cc].T
                off_bf += cc
            else:
                vals = f3[:, off_f:off_f + cc].T
                off_f += cc
            Tfull[(1 << k) + core:1 << (k + 1):NCORES] = vals
    outv = Tfull[unique.astype(np.int64)]
    return outv, res


def kernel(unique, primitives, identity):
    out, _ = _run(unique, primitives, identity)
    return out


if __name__ == "__main__":
    # tiny smoke run (full shapes) — prefer test.py for the real check
    rng = np.random.default_rng(0)
    u = rng.integers(0, 1 << 17, size=131072).astype(np.int32)
    prims = rng.standard_normal((2, DIM, DIM)).astype(np.float32)
    ones = np.ones((1, DIM), np.float32)
    out = kernel(u, prims, ones)
    print("kernel output", out.shape, out.dtype)
